# revision 13
# baseline (speedup 1.0000x reference)
"""CERNN (context-dependent excitatory RNN) Trainium2 kernel.

Reference computation (see problem):
    w_ih_eff = weight_ih * (mask_s1 + mask_v1 + mask_taskid)
    w_hh_eff = weight_hh * intra_area_mask * (|weight_hh| > 0.01)
    h0 = init_hidden fixed-point loop (batch rows identical -> computed host-side, B=1)
    scan t in [0,512): h = 0.8*h + 0.2*relu(x_t @ w_ih_eff.T + h @ w_hh_eff.T + bias)
    hidden = all h states; output = hidden[:, :, 128:192] @ w_out.T

Strategy:
  - Data-parallel over batch: B=256 -> 8 cores x 32.
  - Device computes the T=512 sequential scan.  Matmul operands are fp16
    (tile_position requires a 16-bit dtype; fp16's 10-bit mantissa keeps the
    per-step rounding ~5e-4 and the recurrence is contractive), state and
    PSUM accumulation stay fp32.
  - Hidden state for matmuls kept TRANSPOSED in SBUF as
    h16[i' (128 partitions), c (8 chunks), b (32)], i = 128c+i'.
  - Per step, pre-activation computed in [b, j] orientation via 4 col-tiled
    matmuls per i-chunk (stationary = h16 chunk [128,32], moving = 0.2*W^T
    quarter [128,256] fp16).  The 0.2 (ALPHA) is pre-folded into the weights
    so relu(psum) needs no scaling.  PSUM row 32g+b, col c = pre[b, 256g+c].
  - x-projection + bias folded into the same PSUM accumulation via an
    augmented 54-row input (x_t; 1.0) against [0.2*w_ih_eff^T; 0.2*bias].
  - The [32,1024] -> [128, 8, 32] transpose back to hT is done on PE with 8
    row-tiled fp16 matmuls against a block-diagonal identity (4 PSUM banks,
    2 chunks per bank -> concurrent across row strips).
  - State update fused into DVE scalar_tensor_tensor ops reading PSUM:
      h16' = (h_f32 * 0.8) + rT   (fp16 out; first - gates the next step)
      stg' = (h_f32 * 0.8) + rT   (fp32 out; the exact state / output)
  - Hidden states staged in SBUF for U steps, DMA'd out in 2MB blocks to a
    transposed DRAM buffer; host untransposes and applies the tiny output
    projection (0.5% of FLOPs).
"""

import os
import sys
import time

import numpy as np

for _p in ("/opt/trn_rl_repo", "/root/.axon_site/_ro/trn_rl_repo"):
    if os.path.isdir(_p) and _p not in sys.path:
        sys.path.append(_p)

N_INPUT = 53
N_RNN = 1024
N_OUT = 33
M_START, M_END = 128, 192
ALPHA = np.float32(0.2)
DECAY = np.float32(0.8)
THRES = 0.01
T_FULL = 512
B_FULL = 256
N_CORES = 8
B_SH = B_FULL // N_CORES  # 32
K_AUG = N_INPUT + 1  # x rows + ones row (bias)
NC8 = N_RNN // 128  # 8 chunks of 128

# packed fp16 const layout (columns within the [128, C16] tensor)
OFF_WHH = 0                      # [128, 8*1024]  whh[p, c*1024+j] = 0.2*w_hh_eff[j, 128c+p]
OFF_WIHB = 8 * 1024              # [128, 1024]    rows 0-52: 0.2*w_ih_eff.T; row 53: 0.2*bias
OFF_IDENT = OFF_WIHB + N_RNN     # [128, 32]      block-diagonal identity
OFF_H016 = OFF_IDENT + B_SH      # [128, 8*32]    fp16 h0 (initial matmul state)
C16 = OFF_H016 + NC8 * B_SH

_module_cache = {}


def _ensure_ntff_hook():
    """Install the axon NTFF profiling hook if antenv.axon_hooks is absent
    (dev/profiling only; the grading path never sets CERNN_TRACE)."""
    try:
        from antenv import axon_hooks  # noqa: F401
        return
    except ImportError:
        pass
    import types

    import antenv

    mod = types.ModuleType("antenv.axon_hooks")
    holder = {}
    mod.set_axon_ntff_profile_hook = lambda h: holder.__setitem__("h", h)
    mod.get_axon_ntff_profile_hook = lambda: holder.get("h")
    sys.modules["antenv.axon_hooks"] = mod
    antenv.axon_hooks = mod
    try:
        from trn_agent_boot.trn_boot import _ntff_profile_via_ctypes

        h = _ntff_profile_via_ctypes("/opt/axon/libaxon_pjrt.so")
        if h is not None:
            mod.set_axon_ntff_profile_hook(h)
    except Exception as e:
        print(f"[kernel] ntff hook setup failed: {e}")


def _build_module(T, U):
    """Build the Bass module for a T-step scan with U-step unrolled For_i body."""
    from contextlib import ExitStack

    import concourse.bass as bass
    import concourse.mybir as mybir
    import concourse.tile as tile
    from concourse import bacc

    f32 = mybir.dt.float32
    f16 = mybir.dt.float16
    ITERS = T // U
    assert T % U == 0

    nc = bacc.Bacc("TRN2", target_bir_lowering=False)

    xT_d = nc.dram_tensor("xT", [K_AUG, T, B_SH], f16, kind="ExternalInput")
    c16_d = nc.dram_tensor("c16", [128, C16], f16, kind="ExternalInput")
    h0f_d = nc.dram_tensor("h0f", [128, NC8 * B_SH], f32, kind="ExternalInput")
    hidT_d = nc.dram_tensor("hidT", [128, T, NC8, B_SH], f32, kind="ExternalOutput")

    with tile.TileContext(nc) as tc, ExitStack() as ctx:
        const = ctx.enter_context(tc.tile_pool(name="const", bufs=1))
        xpool = ctx.enter_context(tc.tile_pool(name="xblk", bufs=2))
        stage = ctx.enter_context(tc.tile_pool(name="stage", bufs=2))
        rpool = ctx.enter_context(tc.tile_pool(name="relu", bufs=2))
        pmain = ctx.enter_context(
            tc.tile_pool(name="pmain", bufs=2, space=bass.MemorySpace.PSUM)
        )
        ptr = ctx.enter_context(
            tc.tile_pool(name="ptr", bufs=1, space=bass.MemorySpace.PSUM)
        )

        c16 = const.tile([128, C16], f16)
        hprev = const.tile([128, NC8 * B_SH], f32)
        h16 = const.tile([128, NC8, B_SH], f16)
        nc.sync.dma_start(c16[:], c16_d[:])
        nc.sync.dma_start(hprev[:], h0f_d[:])
        nc.vector.tensor_copy(h16[:], c16[:, OFF_H016 : OFF_H016 + NC8 * B_SH])

        def whh_q(c, g):  # rhs [128, 256] for chunk c, quarter g
            o = OFF_WHH + c * N_RNN + 256 * g
            return c16[:, o : o + 256]

        def wihb_q(g):  # rhs [54, 256] for quarter g
            o = OFF_WIHB + 256 * g
            return c16[0:K_AUG, o : o + 256]

        def ident_g(g):  # [32, 32] identity at partitions 32g
            return c16[32 * g : 32 * g + 32, OFF_IDENT : OFF_IDENT + B_SH]

        def step(u, prevf, xblk, stg):
            """One timestep.  prevf = previous fp32 state AP [128, 8*32]."""
            # full-bank psum tile (start=True pending-zeroes the whole 2KB row)
            psum = pmain.tile([128, 512], f32)
            # x-projection + bias (start accumulation), 4 col-tiled quarters
            for g in range(4):
                nc.tensor.matmul(
                    psum[32 * g : 32 * g + 32, 0:256],
                    xblk[:, u, :],
                    wihb_q(g),
                    start=True,
                    stop=False,
                    tile_position=(0, 32 * g),
                    skip_group_check=(g > 0),
                )
            # recurrent: 8 i-chunks x 4 col groups (stationary = h16 chunk)
            for c in range(NC8):
                for g in range(4):
                    nc.tensor.matmul(
                        psum[32 * g : 32 * g + 32, 0:256],
                        h16[:, c, :],
                        whh_q(c, g),
                        start=False,
                        stop=(c == NC8 - 1),
                        tile_position=(0, 32 * g),
                        skip_group_check=(g > 0),
                    )
            # relu in two column halves so transposes can start earlier;
            # r[32g+b, col] = 0.2*relu(pre[b, 256g+col]) (ALPHA pre-folded)
            r = rpool.tile([128, 256], f16)
            for h in range(2):
                nc.scalar.activation(
                    r[:, 128 * h : 128 * h + 128],
                    psum[:, 128 * h : 128 * h + 128],
                    mybir.ActivationFunctionType.Relu,
                )
            # transpose r back to hT layout: bank g holds chunks {2g, 2g+1};
            # the four banks' matmuls run concurrently (distinct row strips
            # within a round, distinct banks)
            trps = [
                ptr.tile([128, 512], f32, tag=f"trp{g}", name=f"trp{g}")
                for g in range(4)
            ]
            for h in range(2):
                for g in range(4):
                    nc.tensor.matmul(
                        trps[g][:, 32 * h : 32 * h + 32],
                        r[32 * g : 32 * g + 32, 128 * h : 128 * h + 128],
                        ident_g(g),
                        start=(h == 0),
                        stop=(h == 1),
                        tile_position=(32 * g, 0),
                    )
            # blend: fp16 state first (gates next step's matmuls), fp32 after
            h16f = h16[:].rearrange("p c b -> p (c b)")
            for g in range(4):
                nc.vector.scalar_tensor_tensor(
                    out=h16f[:, 64 * g : 64 * g + 64],
                    in0=prevf[:, 64 * g : 64 * g + 64],
                    scalar=float(DECAY),
                    in1=trps[g][:, 0:64],
                    op0=mybir.AluOpType.mult,
                    op1=mybir.AluOpType.add,
                )
            stgf = stg[:].rearrange("p u c b -> p (u c b)")
            for g in range(4):
                nc.vector.scalar_tensor_tensor(
                    out=stgf[:, u * 256 + 64 * g : u * 256 + 64 * g + 64],
                    in0=prevf[:, 64 * g : 64 * g + 64],
                    scalar=float(DECAY),
                    in1=trps[g][:, 0:64],
                    op0=mybir.AluOpType.mult,
                    op1=mybir.AluOpType.add,
                )

        with tc.For_i(0, ITERS) as it:
            xblk = xpool.tile([K_AUG, U, B_SH], f16)
            nc.sync.dma_start(xblk[:], xT_d[:, bass.ts(it, U), :])
            stg = stage.tile([128, U, NC8, B_SH], f32)
            stgf = stg[:].rearrange("p u c b -> p (u c b)")
            for u in range(U):
                prevf = hprev[:] if u == 0 else stgf[:, (u - 1) * 256 : u * 256]
                step(u, prevf, xblk, stg)
            nc.vector.tensor_copy(hprev[:], stg[:, U - 1, :, :])
            nc.sync.dma_start(hidT_d[:, bass.ts(it, U), :, :], stg[:])

    nc.compile()
    return nc


def _prep_host(x, weight_ih, weight_hh, bias, mask_s1, mask_v1, mask_taskid, intra):
    """Host-side: effective weights, init fixed point (B=1)."""
    w_ih_eff = (weight_ih * (mask_s1 + mask_v1 + mask_taskid)).astype(np.float32)
    nonzero = (np.abs(weight_hh) > THRES).astype(np.float32)
    w_hh_eff = (weight_hh * (intra * nonzero)).astype(np.float32)

    def step1(h, x_t):
        pre = x_t @ w_ih_eff.T + h @ w_hh_eff.T + bias
        return DECAY * h + ALPHA * np.maximum(pre, np.float32(0))

    x_init = x[:2, 0:1, :]
    h0 = np.zeros((1, N_RNN), np.float32)
    stable, done = 0, False
    for _ in range(100):
        if done:
            break
        h1 = step1(h0, x_init[0])
        h2 = step1(h1, x_init[1])
        close = bool(np.all(np.abs(h2 - h0) <= 0.1 + 1e-5 * np.abs(h0)))
        stable = stable + 1 if close else 0
        if stable >= 4:
            done = True
        else:
            h0 = h2
    return w_ih_eff, w_hh_eff, h0[0]


def _pack_consts(w_ih_eff, w_hh_eff, bias, h0):
    c16 = np.zeros((128, C16), np.float16)
    whh = (ALPHA * w_hh_eff).T.reshape(NC8, 128, N_RNN).transpose(1, 0, 2)
    c16[:, OFF_WHH : OFF_WHH + NC8 * N_RNN] = whh.reshape(128, NC8 * N_RNN)
    c16[0:N_INPUT, OFF_WIHB : OFF_WIHB + N_RNN] = ALPHA * w_ih_eff.T
    c16[N_INPUT, OFF_WIHB : OFF_WIHB + N_RNN] = ALPHA * bias
    c16[np.arange(128), OFF_IDENT + np.arange(128) % 32] = 1.0
    h0T = np.broadcast_to(h0.reshape(NC8, 128).T[:, :, None], (128, NC8, B_SH))
    c16[:, OFF_H016 : OFF_H016 + NC8 * B_SH] = h0T.reshape(128, NC8 * B_SH)
    h0f = np.ascontiguousarray(h0T.reshape(128, NC8 * B_SH), dtype=np.float32)
    return c16, h0f


def kernel(x, weight_ih, weight_hh, bias, w_out, mask_s1, mask_v1, mask_taskid,
           intra_area_mask):
    x = np.asarray(x, np.float32)
    weight_ih = np.asarray(weight_ih, np.float32)
    weight_hh = np.asarray(weight_hh, np.float32)
    bias = np.asarray(bias, np.float32)
    w_out = np.asarray(w_out, np.float32)
    mask_s1 = np.asarray(mask_s1, np.float32)
    mask_v1 = np.asarray(mask_v1, np.float32)
    mask_taskid = np.asarray(mask_taskid, np.float32)
    intra = np.asarray(intra_area_mask, np.float32)

    T = int(os.environ.get("CERNN_T", T_FULL))
    U = int(os.environ.get("CERNN_U", 16))
    trace = bool(int(os.environ.get("CERNN_TRACE", "0")))

    w_ih_eff, w_hh_eff, h0 = _prep_host(
        x, weight_ih, weight_hh, bias, mask_s1, mask_v1, mask_taskid, intra
    )
    c16, h0f = _pack_consts(w_ih_eff, w_hh_eff, bias, h0)

    key = (T, U)
    if key not in _module_cache:
        _module_cache[key] = _build_module(T, U)
    nc = _module_cache[key]

    from concourse import bass_utils

    if trace:
        _ensure_ntff_hook()

    in_maps = []
    for core in range(N_CORES):
        xs = x[:T, core * B_SH : (core + 1) * B_SH, :]  # [T, 32, 53]
        xT = np.empty((K_AUG, T, B_SH), np.float16)
        xT[:N_INPUT] = xs.transpose(2, 0, 1)
        xT[N_INPUT] = 1.0
        in_maps.append(
            {"xT": np.ascontiguousarray(xT), "c16": c16, "h0f": h0f}
        )

    t0 = time.time()
    res = bass_utils.run_bass_kernel_spmd(
        nc, in_maps, core_ids=list(range(N_CORES)), trace=trace
    )
    wall = time.time() - t0
    if trace:
        print(f"[kernel] exec_time_ns={res.exec_time_ns} mean={res.mean_exec_time_ns} "
              f"wall={wall:.2f}s trace={res.instructions_and_trace[1] if res.instructions_and_trace else None}")
        kernel.last_exec_time_ns = res.exec_time_ns
    kernel.last_wall = wall

    # Gather + untranspose: hidT[i', t, c, b] -> hidden[t, core*32+b, 128c+i']
    hidden = np.empty((T, B_FULL, N_RNN), np.float32)
    for core in range(N_CORES):
        hT = res.results[core]["hidT"]  # [128, T, 8, 32]
        hidden[:, core * B_SH : (core + 1) * B_SH, :] = (
            hT.transpose(1, 3, 2, 0).reshape(T, B_SH, N_RNN)
        )

    output = hidden[:, :, M_START:M_END] @ w_out.T
    return output, hidden


# revision 14
# speedup vs baseline: 1.0322x; 1.0322x over previous
"""CERNN (context-dependent excitatory RNN) Trainium2 kernel.

Reference computation (see problem):
    w_ih_eff = weight_ih * (mask_s1 + mask_v1 + mask_taskid)
    w_hh_eff = weight_hh * intra_area_mask * (|weight_hh| > 0.01)
    h0 = init_hidden fixed-point loop (batch rows identical -> computed host-side, B=1)
    scan t in [0,512): h = 0.8*h + 0.2*relu(x_t @ w_ih_eff.T + h @ w_hh_eff.T + bias)
    hidden = all h states; output = hidden[:, :, 128:192] @ w_out.T

Strategy:
  - Data-parallel over batch: B=256 -> 8 cores x 32.
  - Device computes the T=512 sequential scan.  Matmul operands are fp16
    (tile_position requires a 16-bit dtype; fp16's 10-bit mantissa keeps the
    per-step rounding ~5e-4 and the recurrence is contractive), state and
    PSUM accumulation stay fp32.
  - Hidden state for matmuls kept TRANSPOSED in SBUF as
    h16[i' (128 partitions), c (8 chunks), b (32)], i = 128c+i'.
  - Per step, pre-activation computed in [b, j] orientation via 4 col-tiled
    matmuls per i-chunk (stationary = h16 chunk [128,32], moving = 0.2*W^T
    quarter [128,256] fp16).  The 0.2 (ALPHA) is pre-folded into the weights
    so relu(psum) needs no scaling.  PSUM row 32g+b, col c = pre[b, 256g+c].
  - x-projection + bias folded into the same PSUM accumulation via an
    augmented 54-row input (x_t; 1.0) against [0.2*w_ih_eff^T; 0.2*bias].
  - The [32,1024] -> [128, 8, 32] transpose back to hT is done on PE with 8
    row-tiled fp16 matmuls against a block-diagonal identity (4 PSUM banks,
    2 chunks per bank -> concurrent across row strips).
  - State update fused into DVE scalar_tensor_tensor ops reading PSUM:
      h16' = (h_f32 * 0.8) + rT   (fp16 out; first - gates the next step)
      stg' = (h_f32 * 0.8) + rT   (fp32 out; the exact state / output)
  - Hidden states staged in SBUF for U steps, DMA'd out in 2MB blocks to a
    transposed DRAM buffer; host untransposes and applies the tiny output
    projection (0.5% of FLOPs).
"""

import os
import sys
import time

import numpy as np

for _p in ("/opt/trn_rl_repo", "/root/.axon_site/_ro/trn_rl_repo"):
    if os.path.isdir(_p) and _p not in sys.path:
        sys.path.append(_p)

N_INPUT = 53
N_RNN = 1024
N_OUT = 33
M_START, M_END = 128, 192
ALPHA = np.float32(0.2)
DECAY = np.float32(0.8)
THRES = 0.01
T_FULL = 512
B_FULL = 256
N_CORES = 8
B_SH = B_FULL // N_CORES  # 32
K_AUG = N_INPUT + 1  # x rows + ones row (bias)
NC8 = N_RNN // 128  # 8 chunks of 128

# packed fp16 const layout (columns within the [128, C16] tensor)
OFF_WHH = 0                      # [128, 8*1024]  whh[p, c*1024+j] = 0.2*w_hh_eff[j, 128c+p]
OFF_WIHB = 8 * 1024              # [128, 1024]    rows 0-52: 0.2*w_ih_eff.T; row 53: 0.2*bias
OFF_IDENT = OFF_WIHB + N_RNN     # [128, 32]      block-diagonal identity
OFF_H016 = OFF_IDENT + B_SH      # [128, 8*32]    fp16 h0 (initial matmul state)
C16 = OFF_H016 + NC8 * B_SH

_module_cache = {}


def _ensure_ntff_hook():
    """Install the axon NTFF profiling hook if antenv.axon_hooks is absent
    (dev/profiling only; the grading path never sets CERNN_TRACE)."""
    try:
        from antenv import axon_hooks  # noqa: F401
        return
    except ImportError:
        pass
    import types

    import antenv

    mod = types.ModuleType("antenv.axon_hooks")
    holder = {}
    mod.set_axon_ntff_profile_hook = lambda h: holder.__setitem__("h", h)
    mod.get_axon_ntff_profile_hook = lambda: holder.get("h")
    sys.modules["antenv.axon_hooks"] = mod
    antenv.axon_hooks = mod
    try:
        from trn_agent_boot.trn_boot import _ntff_profile_via_ctypes

        h = _ntff_profile_via_ctypes("/opt/axon/libaxon_pjrt.so")
        if h is not None:
            mod.set_axon_ntff_profile_hook(h)
    except Exception as e:
        print(f"[kernel] ntff hook setup failed: {e}")


def _build_module(T, U):
    """Build the Bass module for a T-step scan with U-step unrolled For_i body."""
    from contextlib import ExitStack

    import concourse.bass as bass
    import concourse.mybir as mybir
    import concourse.tile as tile
    from concourse import bacc

    f32 = mybir.dt.float32
    f16 = mybir.dt.float16
    ITERS = T // U
    assert T % U == 0

    nc = bacc.Bacc("TRN2", target_bir_lowering=False)

    xT_d = nc.dram_tensor("xT", [K_AUG, T, B_SH], f16, kind="ExternalInput")
    c16_d = nc.dram_tensor("c16", [128, C16], f16, kind="ExternalInput")
    h0f_d = nc.dram_tensor("h0f", [128, NC8 * B_SH], f32, kind="ExternalInput")
    hidT_d = nc.dram_tensor("hidT", [128, T, NC8, B_SH], f32, kind="ExternalOutput")

    with tile.TileContext(nc) as tc, ExitStack() as ctx:
        const = ctx.enter_context(tc.tile_pool(name="const", bufs=1))
        xpool = ctx.enter_context(tc.tile_pool(name="xblk", bufs=2))
        stage = ctx.enter_context(tc.tile_pool(name="stage", bufs=2))
        rpool = ctx.enter_context(tc.tile_pool(name="relu", bufs=2))
        pmain = ctx.enter_context(
            tc.tile_pool(name="pmain", bufs=2, space=bass.MemorySpace.PSUM)
        )
        ptr = ctx.enter_context(
            tc.tile_pool(name="ptr", bufs=1, space=bass.MemorySpace.PSUM)
        )

        c16 = const.tile([128, C16], f16)
        hprev = const.tile([128, NC8 * B_SH], f32)
        h16 = const.tile([128, NC8, B_SH], f16)
        nc.sync.dma_start(c16[:], c16_d[:])
        nc.sync.dma_start(hprev[:], h0f_d[:])
        nc.vector.tensor_copy(h16[:], c16[:, OFF_H016 : OFF_H016 + NC8 * B_SH])

        def whh_q(c, g):  # rhs [128, 256] for chunk c, quarter g
            o = OFF_WHH + c * N_RNN + 256 * g
            return c16[:, o : o + 256]

        def wihb_q(g):  # rhs [54, 256] for quarter g
            o = OFF_WIHB + 256 * g
            return c16[0:K_AUG, o : o + 256]

        def ident_g(g):  # [32, 32] identity at partitions 32g
            return c16[32 * g : 32 * g + 32, OFF_IDENT : OFF_IDENT + B_SH]

        def step(u, prevf, xblk, stg):
            """One timestep.  prevf = previous fp32 state AP [128, 8*32]."""
            # full-bank psum tile (start=True pending-zeroes the whole 2KB row)
            psum = pmain.tile([128, 512], f32)
            # x-projection + bias (start accumulation), 4 col-tiled quarters
            for g in range(4):
                nc.tensor.matmul(
                    psum[32 * g : 32 * g + 32, 0:256],
                    xblk[:, u, :],
                    wihb_q(g),
                    start=True,
                    stop=False,
                    tile_position=(0, 32 * g),
                    skip_group_check=(g > 0),
                )
            # recurrent: 8 i-chunks x 4 col groups (stationary = h16 chunk)
            for c in range(NC8):
                for g in range(4):
                    nc.tensor.matmul(
                        psum[32 * g : 32 * g + 32, 0:256],
                        h16[:, c, :],
                        whh_q(c, g),
                        start=False,
                        stop=(c == NC8 - 1),
                        tile_position=(0, 32 * g),
                        skip_group_check=(g > 0),
                    )
            # relu halves run in parallel on ACT and DVE;
            # r[32g+b, col] = 0.2*relu(pre[b, 256g+col]) (ALPHA pre-folded)
            r = rpool.tile([128, 256], f16)
            nc.scalar.activation(
                r[:, 0:128], psum[:, 0:128], mybir.ActivationFunctionType.Relu
            )
            nc.vector.tensor_relu(r[:, 128:256], psum[:, 128:256])
            # transpose r back to hT layout: one 4-bank psum tile, bank g
            # holds chunks {2g, 2g+1}; a round's four matmuls hit distinct
            # row strips and distinct banks -> concurrent
            trp = ptr.tile([128, 4, 512], f32)
            for h in range(2):
                for g in range(4):
                    nc.tensor.matmul(
                        trp[:, g, 32 * h : 32 * h + 32],
                        r[32 * g : 32 * g + 32, 128 * h : 128 * h + 128],
                        ident_g(g),
                        start=(h == 0),
                        stop=(h == 1),
                        tile_position=(32 * g, 0),
                    )
            # blend in two fused DVE ops: fp16 state first (gates the next
            # step's matmuls), fp32 state/output second
            rt = trp[:, :, 0:64]  # [128, 4, 64] strided view of all 8 chunks
            h16v = h16[:].rearrange("p (g t) b -> p g (t b)", g=4)
            prevv = prevf.rearrange("p (g x) -> p g x", g=4)
            nc.vector.scalar_tensor_tensor(
                out=h16v,
                in0=prevv,
                scalar=float(DECAY),
                in1=rt,
                op0=mybir.AluOpType.mult,
                op1=mybir.AluOpType.add,
            )
            stgv = stg[:, u, :, :].rearrange("p (g t) b -> p g (t b)", g=4)
            nc.vector.scalar_tensor_tensor(
                out=stgv,
                in0=prevv,
                scalar=float(DECAY),
                in1=rt,
                op0=mybir.AluOpType.mult,
                op1=mybir.AluOpType.add,
            )

        with tc.For_i(0, ITERS) as it:
            xblk = xpool.tile([K_AUG, U, B_SH], f16)
            nc.sync.dma_start(xblk[:], xT_d[:, bass.ts(it, U), :])
            stg = stage.tile([128, U, NC8, B_SH], f32)
            stgf = stg[:].rearrange("p u c b -> p (u c b)")
            for u in range(U):
                prevf = hprev[:] if u == 0 else stgf[:, (u - 1) * 256 : u * 256]
                step(u, prevf, xblk, stg)
            nc.vector.tensor_copy(hprev[:], stg[:, U - 1, :, :])
            nc.sync.dma_start(hidT_d[:, bass.ts(it, U), :, :], stg[:])

    nc.compile()
    return nc


def _prep_host(x, weight_ih, weight_hh, bias, mask_s1, mask_v1, mask_taskid, intra):
    """Host-side: effective weights, init fixed point (B=1)."""
    w_ih_eff = (weight_ih * (mask_s1 + mask_v1 + mask_taskid)).astype(np.float32)
    nonzero = (np.abs(weight_hh) > THRES).astype(np.float32)
    w_hh_eff = (weight_hh * (intra * nonzero)).astype(np.float32)

    def step1(h, x_t):
        pre = x_t @ w_ih_eff.T + h @ w_hh_eff.T + bias
        return DECAY * h + ALPHA * np.maximum(pre, np.float32(0))

    x_init = x[:2, 0:1, :]
    h0 = np.zeros((1, N_RNN), np.float32)
    stable, done = 0, False
    for _ in range(100):
        if done:
            break
        h1 = step1(h0, x_init[0])
        h2 = step1(h1, x_init[1])
        close = bool(np.all(np.abs(h2 - h0) <= 0.1 + 1e-5 * np.abs(h0)))
        stable = stable + 1 if close else 0
        if stable >= 4:
            done = True
        else:
            h0 = h2
    return w_ih_eff, w_hh_eff, h0[0]


def _pack_consts(w_ih_eff, w_hh_eff, bias, h0):
    c16 = np.zeros((128, C16), np.float16)
    whh = (ALPHA * w_hh_eff).T.reshape(NC8, 128, N_RNN).transpose(1, 0, 2)
    c16[:, OFF_WHH : OFF_WHH + NC8 * N_RNN] = whh.reshape(128, NC8 * N_RNN)
    c16[0:N_INPUT, OFF_WIHB : OFF_WIHB + N_RNN] = ALPHA * w_ih_eff.T
    c16[N_INPUT, OFF_WIHB : OFF_WIHB + N_RNN] = ALPHA * bias
    c16[np.arange(128), OFF_IDENT + np.arange(128) % 32] = 1.0
    h0T = np.broadcast_to(h0.reshape(NC8, 128).T[:, :, None], (128, NC8, B_SH))
    c16[:, OFF_H016 : OFF_H016 + NC8 * B_SH] = h0T.reshape(128, NC8 * B_SH)
    h0f = np.ascontiguousarray(h0T.reshape(128, NC8 * B_SH), dtype=np.float32)
    return c16, h0f


def kernel(x, weight_ih, weight_hh, bias, w_out, mask_s1, mask_v1, mask_taskid,
           intra_area_mask):
    x = np.asarray(x, np.float32)
    weight_ih = np.asarray(weight_ih, np.float32)
    weight_hh = np.asarray(weight_hh, np.float32)
    bias = np.asarray(bias, np.float32)
    w_out = np.asarray(w_out, np.float32)
    mask_s1 = np.asarray(mask_s1, np.float32)
    mask_v1 = np.asarray(mask_v1, np.float32)
    mask_taskid = np.asarray(mask_taskid, np.float32)
    intra = np.asarray(intra_area_mask, np.float32)

    T = int(os.environ.get("CERNN_T", T_FULL))
    U = int(os.environ.get("CERNN_U", 16))
    trace = bool(int(os.environ.get("CERNN_TRACE", "0")))

    w_ih_eff, w_hh_eff, h0 = _prep_host(
        x, weight_ih, weight_hh, bias, mask_s1, mask_v1, mask_taskid, intra
    )
    c16, h0f = _pack_consts(w_ih_eff, w_hh_eff, bias, h0)

    key = (T, U)
    if key not in _module_cache:
        _module_cache[key] = _build_module(T, U)
    nc = _module_cache[key]

    from concourse import bass_utils

    if trace:
        _ensure_ntff_hook()

    in_maps = []
    for core in range(N_CORES):
        xs = x[:T, core * B_SH : (core + 1) * B_SH, :]  # [T, 32, 53]
        xT = np.empty((K_AUG, T, B_SH), np.float16)
        xT[:N_INPUT] = xs.transpose(2, 0, 1)
        xT[N_INPUT] = 1.0
        in_maps.append(
            {"xT": np.ascontiguousarray(xT), "c16": c16, "h0f": h0f}
        )

    t0 = time.time()
    res = bass_utils.run_bass_kernel_spmd(
        nc, in_maps, core_ids=list(range(N_CORES)), trace=trace
    )
    wall = time.time() - t0
    if trace:
        print(f"[kernel] exec_time_ns={res.exec_time_ns} mean={res.mean_exec_time_ns} "
              f"wall={wall:.2f}s trace={res.instructions_and_trace[1] if res.instructions_and_trace else None}")
        kernel.last_exec_time_ns = res.exec_time_ns
    kernel.last_wall = wall

    # Gather + untranspose: hidT[i', t, c, b] -> hidden[t, core*32+b, 128c+i']
    hidden = np.empty((T, B_FULL, N_RNN), np.float32)
    for core in range(N_CORES):
        hT = res.results[core]["hidT"]  # [128, T, 8, 32]
        hidden[:, core * B_SH : (core + 1) * B_SH, :] = (
            hT.transpose(1, 3, 2, 0).reshape(T, B_SH, N_RNN)
        )

    output = hidden[:, :, M_START:M_END] @ w_out.T
    return output, hidden


# revision 17
# speedup vs baseline: 1.0507x; 1.0179x over previous
"""CERNN (context-dependent excitatory RNN) Trainium2 kernel.

Reference computation (see problem):
    w_ih_eff = weight_ih * (mask_s1 + mask_v1 + mask_taskid)
    w_hh_eff = weight_hh * intra_area_mask * (|weight_hh| > 0.01)
    h0 = init_hidden fixed-point loop (batch rows identical -> computed host-side, B=1)
    scan t in [0,512): h = 0.8*h + 0.2*relu(x_t @ w_ih_eff.T + h @ w_hh_eff.T + bias)
    hidden = all h states; output = hidden[:, :, 128:192] @ w_out.T

Strategy:
  - Data-parallel over batch: B=256 -> 8 cores x 32.
  - Device computes the T=512 sequential scan.  Matmul operands are fp16
    (tile_position requires a 16-bit dtype; fp16's 10-bit mantissa keeps the
    per-step rounding ~5e-4 and the recurrence is contractive), state and
    PSUM accumulation stay fp32.
  - Hidden state for matmuls kept TRANSPOSED in SBUF as
    h16[i' (128 partitions), c (8 chunks), b (32)], i = 128c+i'.
  - Per step, pre-activation computed in [b, j] orientation via 4 col-tiled
    matmuls per i-chunk (stationary = h16 chunk [128,32], moving = 0.2*W^T
    quarter [128,256] fp16).  The 0.2 (ALPHA) is pre-folded into the weights
    so relu(psum) needs no scaling.  PSUM row 32g+b, col c = pre[b, 256g+c].
  - x-projection + bias folded into the same PSUM accumulation via an
    augmented 54-row input (x_t; 1.0) against [0.2*w_ih_eff^T; 0.2*bias].
  - The [32,1024] -> [128, 8, 32] transpose back to hT is done on PE with 8
    row-tiled fp16 matmuls against a block-diagonal identity (4 PSUM banks,
    2 chunks per bank -> concurrent across row strips).
  - State update fused into DVE scalar_tensor_tensor ops reading PSUM:
      h16' = (h_f32 * 0.8) + rT   (fp16 out; first - gates the next step)
      stg' = (h_f32 * 0.8) + rT   (fp32 out; the exact state / output)
  - Hidden states staged in SBUF for U steps, DMA'd out in 2MB blocks to a
    transposed DRAM buffer; host untransposes and applies the tiny output
    projection (0.5% of FLOPs).
"""

import os
import sys
import time

import numpy as np

for _p in ("/opt/trn_rl_repo", "/root/.axon_site/_ro/trn_rl_repo"):
    if os.path.isdir(_p) and _p not in sys.path:
        sys.path.append(_p)

N_INPUT = 53
N_RNN = 1024
N_OUT = 33
M_START, M_END = 128, 192
ALPHA = np.float32(0.2)
DECAY = np.float32(0.8)
THRES = 0.01
T_FULL = 512
B_FULL = 256
N_CORES = 8
B_SH = B_FULL // N_CORES  # 32
K_AUG = N_INPUT + 1  # x rows + ones row (bias)
NC8 = N_RNN // 128  # 8 chunks of 128

# packed fp16 const layout (columns within the [128, C16] tensor)
OFF_WHH = 0                      # [128, 8*1024]  whh[p, c*1024+j] = 0.2*w_hh_eff[j, 128c+p]
OFF_WIHB = 8 * 1024              # [128, 1024]    rows 0-52: 0.2*w_ih_eff.T; row 53: 0.2*bias
OFF_IDENT = OFF_WIHB + N_RNN     # [128, 32]      block-diagonal identity
OFF_H016 = OFF_IDENT + B_SH      # [128, 8*32]    fp16 h0 (initial matmul state)
C16 = OFF_H016 + NC8 * B_SH

_module_cache = {}


def _ensure_ntff_hook():
    """Install the axon NTFF profiling hook if antenv.axon_hooks is absent
    (dev/profiling only; the grading path never sets CERNN_TRACE)."""
    try:
        from antenv import axon_hooks  # noqa: F401
        return
    except ImportError:
        pass
    import types

    import antenv

    mod = types.ModuleType("antenv.axon_hooks")
    holder = {}
    mod.set_axon_ntff_profile_hook = lambda h: holder.__setitem__("h", h)
    mod.get_axon_ntff_profile_hook = lambda: holder.get("h")
    sys.modules["antenv.axon_hooks"] = mod
    antenv.axon_hooks = mod
    try:
        from trn_agent_boot.trn_boot import _ntff_profile_via_ctypes

        h = _ntff_profile_via_ctypes("/opt/axon/libaxon_pjrt.so")
        if h is not None:
            mod.set_axon_ntff_profile_hook(h)
    except Exception as e:
        print(f"[kernel] ntff hook setup failed: {e}")


def _build_module(T, U):
    """Build the Bass module for a T-step scan with U-step unrolled For_i body."""
    from contextlib import ExitStack

    import concourse.bass as bass
    import concourse.mybir as mybir
    import concourse.tile as tile
    from concourse import bacc

    f32 = mybir.dt.float32
    f16 = mybir.dt.float16
    ITERS = T // U
    assert T % U == 0

    nc = bacc.Bacc("TRN2", target_bir_lowering=False)

    xT_d = nc.dram_tensor("xT", [K_AUG, T, B_SH], f16, kind="ExternalInput")
    c16_d = nc.dram_tensor("c16", [128, C16], f16, kind="ExternalInput")
    h0f_d = nc.dram_tensor("h0f", [128, NC8 * B_SH], f32, kind="ExternalInput")
    hidT_d = nc.dram_tensor("hidT", [128, T, NC8, B_SH], f32, kind="ExternalOutput")

    with tile.TileContext(nc) as tc, ExitStack() as ctx:
        const = ctx.enter_context(tc.tile_pool(name="const", bufs=1))
        xpool = ctx.enter_context(tc.tile_pool(name="xblk", bufs=2))
        stage = ctx.enter_context(tc.tile_pool(name="stage", bufs=2))
        rpool = ctx.enter_context(tc.tile_pool(name="relu", bufs=2))
        pmain = ctx.enter_context(
            tc.tile_pool(name="pmain", bufs=3, space=bass.MemorySpace.PSUM)
        )
        ptr = ctx.enter_context(
            tc.tile_pool(name="ptr", bufs=1, space=bass.MemorySpace.PSUM)
        )

        c16 = const.tile([128, C16], f16)
        hprev = const.tile([128, NC8 * B_SH], f32)
        h16 = const.tile([128, NC8, B_SH], f16)
        nc.sync.dma_start(c16[:], c16_d[:])
        nc.sync.dma_start(hprev[:], h0f_d[:])
        nc.vector.tensor_copy(h16[:], c16[:, OFF_H016 : OFF_H016 + NC8 * B_SH])

        def whh_q(c, g):  # rhs [128, 256] for chunk c, quarter g
            o = OFF_WHH + c * N_RNN + 256 * g
            return c16[:, o : o + 256]

        def wihb_q(g):  # rhs [54, 256] for quarter g
            o = OFF_WIHB + 256 * g
            return c16[0:K_AUG, o : o + 256]

        def ident_g(g):  # [32, 32] identity at partitions 32g
            return c16[32 * g : 32 * g + 32, OFF_IDENT : OFF_IDENT + B_SH]

        def step(u, prevf, xblk, stg):
            """One timestep.  prevf = previous fp32 state AP [128, 8*32]."""
            # full-bank psum tile (start=True pending-zeroes the whole 2KB row)
            psum = pmain.tile([128, 512], f32)
            # x-projection + bias (start accumulation), 4 col-tiled quarters
            for g in range(4):
                nc.tensor.matmul(
                    psum[32 * g : 32 * g + 32, 0:256],
                    xblk[:, u, :],
                    wihb_q(g),
                    start=True,
                    stop=False,
                    tile_position=(0, 32 * g),
                    skip_group_check=(g > 0),
                )
            # recurrent: 8 i-chunks x 4 col groups (stationary = h16 chunk).
            # Chunk order matches availability from the previous step's
            # transpose rounds (even chunks land first).
            order = (0, 2, 4, 6, 1, 3, 5, 7)
            for ci, c in enumerate(order):
                for g in range(4):
                    nc.tensor.matmul(
                        psum[32 * g : 32 * g + 32, 0:256],
                        h16[:, c, :],
                        whh_q(c, g),
                        start=False,
                        stop=(ci == NC8 - 1),
                        tile_position=(0, 32 * g),
                        skip_group_check=(g > 0),
                    )
            # relu halves run in parallel on ACT and DVE;
            # r[32g+b, col] = 0.2*relu(pre[b, 256g+col]) (ALPHA pre-folded)
            r = rpool.tile([128, 256], f16)
            nc.scalar.activation(
                r[:, 0:128], psum[:, 0:128], mybir.ActivationFunctionType.Relu
            )
            nc.vector.tensor_relu(r[:, 128:256], psum[:, 128:256])
            # transpose r back to hT layout: one 4-bank psum tile, bank g
            # holds chunks {2g, 2g+1}; a round's four matmuls hit distinct
            # row strips and distinct banks -> concurrent
            trp = ptr.tile([128, 4, 512], f32)
            for h in range(2):
                for g in range(4):
                    nc.tensor.matmul(
                        trp[:, g, 32 * h : 32 * h + 32],
                        r[32 * g : 32 * g + 32, 128 * h : 128 * h + 128],
                        ident_g(g),
                        start=(h == 0),
                        stop=(h == 1),
                        tile_position=(32 * g, 0),
                    )
            # blend per transpose round so even chunks unblock the next
            # step's first waves early: fp16 state first (gates matmuls),
            # fp32 state/output second
            h16v = h16[:].rearrange("p (g h) b -> p h g b", h=2)
            prevv = prevf.rearrange("p (g h b) -> p h g b", h=2, b=B_SH)
            stgv = stg[:, u, :, :].rearrange("p (g h) b -> p h g b", h=2)
            for h in range(2):
                nc.vector.scalar_tensor_tensor(
                    out=h16v[:, h],
                    in0=prevv[:, h],
                    scalar=float(DECAY),
                    in1=trp[:, :, 32 * h : 32 * h + 32],
                    op0=mybir.AluOpType.mult,
                    op1=mybir.AluOpType.add,
                )
            for h in range(2):
                nc.vector.scalar_tensor_tensor(
                    out=stgv[:, h],
                    in0=prevv[:, h],
                    scalar=float(DECAY),
                    in1=trp[:, :, 32 * h : 32 * h + 32],
                    op0=mybir.AluOpType.mult,
                    op1=mybir.AluOpType.add,
                )

        with tc.For_i(0, ITERS) as it:
            xblk = xpool.tile([K_AUG, U, B_SH], f16)
            nc.sync.dma_start(xblk[:], xT_d[:, bass.ts(it, U), :])
            stg = stage.tile([128, U, NC8, B_SH], f32)
            stgf = stg[:].rearrange("p u c b -> p (u c b)")
            for u in range(U):
                prevf = hprev[:] if u == 0 else stgf[:, (u - 1) * 256 : u * 256]
                step(u, prevf, xblk, stg)
            nc.vector.tensor_copy(hprev[:], stg[:, U - 1, :, :])
            nc.sync.dma_start(hidT_d[:, bass.ts(it, U), :, :], stg[:])

    nc.compile()
    return nc


def _prep_host(x, weight_ih, weight_hh, bias, mask_s1, mask_v1, mask_taskid, intra):
    """Host-side: effective weights, init fixed point (B=1)."""
    w_ih_eff = (weight_ih * (mask_s1 + mask_v1 + mask_taskid)).astype(np.float32)
    nonzero = (np.abs(weight_hh) > THRES).astype(np.float32)
    w_hh_eff = (weight_hh * (intra * nonzero)).astype(np.float32)

    def step1(h, x_t):
        pre = x_t @ w_ih_eff.T + h @ w_hh_eff.T + bias
        return DECAY * h + ALPHA * np.maximum(pre, np.float32(0))

    x_init = x[:2, 0:1, :]
    h0 = np.zeros((1, N_RNN), np.float32)
    stable, done = 0, False
    for _ in range(100):
        if done:
            break
        h1 = step1(h0, x_init[0])
        h2 = step1(h1, x_init[1])
        close = bool(np.all(np.abs(h2 - h0) <= 0.1 + 1e-5 * np.abs(h0)))
        stable = stable + 1 if close else 0
        if stable >= 4:
            done = True
        else:
            h0 = h2
    return w_ih_eff, w_hh_eff, h0[0]


def _pack_consts(w_ih_eff, w_hh_eff, bias, h0):
    c16 = np.zeros((128, C16), np.float16)
    whh = (ALPHA * w_hh_eff).T.reshape(NC8, 128, N_RNN).transpose(1, 0, 2)
    c16[:, OFF_WHH : OFF_WHH + NC8 * N_RNN] = whh.reshape(128, NC8 * N_RNN)
    c16[0:N_INPUT, OFF_WIHB : OFF_WIHB + N_RNN] = ALPHA * w_ih_eff.T
    c16[N_INPUT, OFF_WIHB : OFF_WIHB + N_RNN] = ALPHA * bias
    c16[np.arange(128), OFF_IDENT + np.arange(128) % 32] = 1.0
    h0T = np.broadcast_to(h0.reshape(NC8, 128).T[:, :, None], (128, NC8, B_SH))
    c16[:, OFF_H016 : OFF_H016 + NC8 * B_SH] = h0T.reshape(128, NC8 * B_SH)
    h0f = np.ascontiguousarray(h0T.reshape(128, NC8 * B_SH), dtype=np.float32)
    return c16, h0f


def kernel(x, weight_ih, weight_hh, bias, w_out, mask_s1, mask_v1, mask_taskid,
           intra_area_mask):
    x = np.asarray(x, np.float32)
    weight_ih = np.asarray(weight_ih, np.float32)
    weight_hh = np.asarray(weight_hh, np.float32)
    bias = np.asarray(bias, np.float32)
    w_out = np.asarray(w_out, np.float32)
    mask_s1 = np.asarray(mask_s1, np.float32)
    mask_v1 = np.asarray(mask_v1, np.float32)
    mask_taskid = np.asarray(mask_taskid, np.float32)
    intra = np.asarray(intra_area_mask, np.float32)

    T = int(os.environ.get("CERNN_T", T_FULL))
    U = int(os.environ.get("CERNN_U", 16))
    trace = bool(int(os.environ.get("CERNN_TRACE", "0")))

    w_ih_eff, w_hh_eff, h0 = _prep_host(
        x, weight_ih, weight_hh, bias, mask_s1, mask_v1, mask_taskid, intra
    )
    c16, h0f = _pack_consts(w_ih_eff, w_hh_eff, bias, h0)

    key = (T, U)
    if key not in _module_cache:
        _module_cache[key] = _build_module(T, U)
    nc = _module_cache[key]

    from concourse import bass_utils

    if trace:
        _ensure_ntff_hook()

    in_maps = []
    for core in range(N_CORES):
        xs = x[:T, core * B_SH : (core + 1) * B_SH, :]  # [T, 32, 53]
        xT = np.empty((K_AUG, T, B_SH), np.float16)
        xT[:N_INPUT] = xs.transpose(2, 0, 1)
        xT[N_INPUT] = 1.0
        in_maps.append(
            {"xT": np.ascontiguousarray(xT), "c16": c16, "h0f": h0f}
        )

    t0 = time.time()
    res = bass_utils.run_bass_kernel_spmd(
        nc, in_maps, core_ids=list(range(N_CORES)), trace=trace
    )
    wall = time.time() - t0
    if trace:
        print(f"[kernel] exec_time_ns={res.exec_time_ns} mean={res.mean_exec_time_ns} "
              f"wall={wall:.2f}s trace={res.instructions_and_trace[1] if res.instructions_and_trace else None}")
        kernel.last_exec_time_ns = res.exec_time_ns
    kernel.last_wall = wall

    # Gather + untranspose: hidT[i', t, c, b] -> hidden[t, core*32+b, 128c+i']
    hidden = np.empty((T, B_FULL, N_RNN), np.float32)
    for core in range(N_CORES):
        hT = res.results[core]["hidT"]  # [128, T, 8, 32]
        hidden[:, core * B_SH : (core + 1) * B_SH, :] = (
            hT.transpose(1, 3, 2, 0).reshape(T, B_SH, N_RNN)
        )

    output = hidden[:, :, M_START:M_END] @ w_out.T
    return output, hidden
